# revision 1
# baseline (speedup 1.0000x reference)
"""Trainium2 Bass kernel for DifferentiableDLT (batched weighted-DLT homography fit).

Contract: kernel(**inputs) takes FULL inputs
    flow (64, 2, 320, 576) f32, mask (64, 1, 320, 576) f32, img_h, img_w
and returns the FULL output (64, 3, 3) f32.

Strategy (pure data parallel, 8 batches/core x 8 cores):
  The 1024 sample points form a fixed separable 32x32 grid, so bilinear
  sampling touches only 64 of 320 rows (32 pairs of adjacent rows) and 64 of
  576 columns.  Per core we:
    1. dma_gather the 64 needed rows (pair vectors for flow, single rows for
       mask) from HBM directly into a partition-optimal SBUF layout.
    2. Select the 64 needed columns with 9 uniform-stride-run copies (DVE).
    3. Bilinear lerp in y then x (DVE tensor ops with constant weight tiles).
    4. One PE transpose puts points on partitions / (tile, batch) on free.
    5. Hartley-normalize dst points; build weighted feature products D =
       [w, w*p, w*q, w*(p^2+q^2)]; the 24 moments that fill A^T A | A^T b are
       C^T @ D with C the constant source-point feature matrix (PE matmul).
    6. Assemble the 8x9 augmented normal equations per batch (batch on
       partitions) and solve with unpivoted Gauss-Jordan (SPD + eps*I).
    7. Denormalize H, sign/scale fix, support gate, DMA out (8,3,3).
"""

import dataclasses
import math
import numpy as np

import concourse.bass as bass
import concourse.bacc as bacc
import concourse.mybir as mybir
from concourse import tile, library_config
from concourse import bass_utils

F32 = mybir.dt.float32
I16 = mybir.dt.int16
ALU = mybir.AluOpType
ACTF = mybir.ActivationFunctionType

NCORES = 8
BPC = 8          # batches per core
HF, WF = 320, 576
NG = 32          # grid is NG x NG points
NPTS = NG * NG
EPS = 1e-6

# ---------------------------------------------------------------------------
# host-side constant computation
# ---------------------------------------------------------------------------


def _grid_1d(size, n):
    m = int(size * 0.05)
    return np.linspace(m, size - m - 1, n, dtype=np.float32)


def _segments(x0):
    """Maximal uniform-step segments (start, len, step) covering x0."""
    segs = []
    i = 0
    n = len(x0)
    while i < n:
        if i == n - 1:
            segs.append((i, 1, 1))
            break
        st = x0[i + 1] - x0[i]
        j = i + 1
        while j + 1 < n and x0[j + 1] - x0[j] == st:
            j += 1
        segs.append((i, j - i + 1, int(st)))
        i = j + 1
    return segs


def _wrap16(idxlist, nslots):
    """dma_gather/ap_gather index wrapping: list pos k -> partition k%16,
    slot k//16, replicated across the 8 gpsimd cores (16-partition groups)."""
    base = np.zeros((16, nslots), np.int16)
    for k, v in enumerate(idxlist):
        base[k % 16, k // 16] = v
    return np.tile(base, (8, 1))


class _Consts:
    def __init__(self, img_h, img_w):
        ys = _grid_1d(HF, NG)
        xs = _grid_1d(WF, NG)
        y0 = np.floor(ys).astype(np.int64)
        x0 = np.floor(xs).astype(np.int64)
        wy = (ys.astype(np.float64) - y0)
        wx = (xs.astype(np.float64) - x0)
        self.segs = _segments(x0)
        self.x0 = x0
        self.sx = float(np.float32((img_w - 1) / max(WF - 1, 1)))
        self.sy = float(np.float32((img_h - 1) / max(HF - 1, 1)))

        # grid points: n = j*NG + i -> (x=xs[i], y=ys[j])
        j = np.arange(NPTS) // NG
        i = np.arange(NPTS) % NG
        gx = xs.astype(np.float64)[i]
        gy = ys.astype(np.float64)[j]

        # constant Hartley normalization of the source points (image coords)
        sxi = gx * self.sx
        syi = gy * self.sy
        mx, my = sxi.mean(), syi.mean()
        cxs, cys = sxi - mx, syi - my
        s_src = max(np.sqrt(cxs * cxs + cys * cys).mean() / math.sqrt(2.0), 1e-8)
        u = cxs / s_src
        v = cys / s_src
        # T_src = [[1/s,0,-mx/s],[0,1/s,-my/s],[0,0,1]] immediates
        self.a_ts = float(np.float32(1.0 / s_src))
        self.c_ts = float(np.float32(-mx / s_src))
        self.d_ts = float(np.float32(-my / s_src))

        # ---- dma_gather index tables ----
        # flow: 512 pair vectors; v = s*128 + p, p = kd*16 + (c*8+b), k = kd*4+s
        self.giF = []
        for h in range(2):
            fidx = np.zeros(256, np.int64)
            for vv in range(256):
                s = 2 * h + vv // 128
                p = vv % 128
                kd, bc = p // 16, p % 16
                c, b = bc // 8, bc % 8
                k = kd * 4 + s
                fidx[vv] = (b * 2 + c) * HF + y0[k]
            self.giF.append(_wrap16(fidx, 16))
        # mask: 512 single-row vectors; p = a*64 + kd*8 + b, k = kd*4+s
        midx = np.zeros(512, np.int64)
        for vv in range(512):
            s = vv // 128
            p = vv % 128
            a = p // 64
            r = p % 64
            kd, b = r // 8, r % 8
            k = kd * 4 + s
            midx[vv] = b * HF + y0[k] + a
        self.giM = _wrap16(midx, 32)

        # ---- interp weight tiles ----
        # flow: partition p=(kd,bc'), free = s*64 + i*2 + c2 ; weight wy[kd*4+s]
        kd_p = np.arange(128) // 16
        s_f = np.arange(256) // 64
        self.WYF = np.asarray(
            wy[kd_p[:, None] * 4 + s_f[None, :]], np.float32
        )  # (128, 256)
        # mask: partition p=(kd,b) in [0,64), same free layout
        kd_m = np.arange(64) // 8
        self.WYM = np.asarray(wy[kd_m[:, None] * 4 + s_f[None, :]], np.float32)
        # x weights: free = s*32 + i
        i_f = np.arange(128) % 32
        self.WXF = np.tile(np.asarray(wx[i_f], np.float32)[None, :], (128, 1))

        # ---- point-feature matrix C6 (128, 8*6): F=[uu, uv, u, vv, v, 1] ----
        feats = np.stack([u * u, u * v, u, v * v, v, np.ones_like(u)], -1)  # (N,6)
        self.C6 = np.ascontiguousarray(
            feats.reshape(8, 128, 6).transpose(1, 0, 2).reshape(128, 48)
        ).astype(np.float32)

        # ---- fused transpose-scale + grid-offset matmul constants ----
        # psF[f, j] = sampF[j, f] * s(c(j)) + grid(f, j), with j = kd*16+c*8+b
        # grid(f, j) = c==0 ? xs[f%32]*sx : ys[4*kd + f//32]*sy
        jj = np.arange(128)
        kd_j = jj // 16
        c_j = (jj % 16) // 8
        sxy = np.where(c_j == 0, self.sx, self.sy).astype(np.float64)
        self.SXYD = (np.eye(128) * sxy[None, :]).astype(np.float32)
        ff = np.arange(128)
        G5 = np.zeros((5, 128), np.float64)
        GR5 = np.zeros((5, 128), np.float64)
        for sp in range(4):
            G5[sp] = (ff // 32 == sp).astype(np.float64)
            GR5[sp] = np.where(c_j == 1, ys.astype(np.float64)[4 * kd_j + sp] * self.sy, 0.0)
        G5[4] = xs.astype(np.float64)[ff % 32]
        GR5[4] = np.where(c_j == 0, self.sx, 0.0)
        self.G5 = G5.astype(np.float32)
        self.GR5 = GR5.astype(np.float32)

        self.IDN = np.eye(128, dtype=np.float32)

        # ---- E matrices: AUG[r*9+c] = sum_q sum_m EQ[q][m, r*9+c] * Mq[m] ----
        E = np.zeros((4, 6, 72), np.float64)
        sym = [[0, 1, 2], [1, 3, 4], [2, 4, 5]]
        for r in range(3):
            for c in range(3):
                m = sym[r][c]
                E[0, m, r * 9 + c] += 1
                E[0, m, (r + 3) * 9 + (c + 3)] += 1
        cr = [[0, 1], [1, 3], [2, 4]]
        for q, r0 in ((1, 0), (2, 3)):
            for r in range(3):
                for c2 in range(2):
                    m = cr[r][c2]
                    E[q, m, (r0 + r) * 9 + 6 + c2] += -1
                    E[q, m, (6 + c2) * 9 + (r0 + r)] += -1
            for r, m in ((0, 2), (1, 4), (2, 5)):
                E[q, m, (r0 + r) * 9 + 8] += 1
        rb = [[0, 1], [1, 3]]
        for r in range(2):
            for c2 in range(2):
                E[3, rb[r][c2], (6 + r) * 9 + 6 + c2] += 1
        E[3, 2, 6 * 9 + 8] += -1
        E[3, 4, 7 * 9 + 8] += -1
        self.EQ = np.ascontiguousarray(
            E.transpose(1, 0, 2).reshape(6, 288)
        ).astype(np.float32)


# ---------------------------------------------------------------------------
# device program
# ---------------------------------------------------------------------------


def _rows_view(ap, nrows, elem):
    """Overlapping rows view of a DRAM tensor: [(WF, nrows), (1, elem)]."""
    flat = ap.rearrange("b c h w -> (b c h w)").unsqueeze(0)
    return dataclasses.replace(flat, ap=[[WF, nrows], [1, elem]])


def _build_program(cc: _Consts):
    nc = bacc.Bacc("TRN2", target_bir_lowering=False, debug=False,
                   num_swdge_queues=2)

    flow = nc.dram_tensor("flow", [BPC, 2, HF, WF], F32, kind="ExternalInput")
    mask = nc.dram_tensor("mask", [BPC, 1, HF, WF], F32, kind="ExternalInput")
    giF0 = nc.dram_tensor("giF0", [128, 16], I16, kind="ExternalInput")
    giF1 = nc.dram_tensor("giF1", [128, 16], I16, kind="ExternalInput")
    giM = nc.dram_tensor("giM", [128, 32], I16, kind="ExternalInput")
    WYF = nc.dram_tensor("WYF", [128, 256], F32, kind="ExternalInput")
    WYM = nc.dram_tensor("WYM", [64, 256], F32, kind="ExternalInput")
    WXF = nc.dram_tensor("WXF", [128, 128], F32, kind="ExternalInput")
    C6 = nc.dram_tensor("C6", [128, 48], F32, kind="ExternalInput")
    SXYD = nc.dram_tensor("SXYD", [128, 128], F32, kind="ExternalInput")
    G5 = nc.dram_tensor("G5", [5, 128], F32, kind="ExternalInput")
    GR5 = nc.dram_tensor("GR5", [5, 128], F32, kind="ExternalInput")
    IDN = nc.dram_tensor("IDN", [128, 128], F32, kind="ExternalInput")
    EQ = nc.dram_tensor("EQ", [6, 288], F32, kind="ExternalInput")
    Hout = nc.dram_tensor("H", [BPC, 3, 3], F32, kind="ExternalOutput")

    V = nc.vector
    A = nc.scalar
    T = nc.tensor
    G = nc.gpsimd
    S = nc.sync

    with tile.TileContext(nc) as tc:
        with (
            tc.tile_pool(name="sb", bufs=1) as pool,
            tc.tile_pool(name="ps", bufs=1, space="PSUM") as psp,
        ):
            # ---------------- constants in ----------------
            def cin(name, src, shape, dtype=F32):
                t = pool.tile(list(shape), dtype, tag=name)
                S.dma_start(t[tuple(slice(0, s) for s in shape)], src[:])
                return t

            # index tables first, on the vector engine's HWDGE queue so the
            # gathers can launch without queueing behind the other constants
            giF0_t = pool.tile([128, 16], I16, tag="giF0")
            A.dma_start(giF0_t[:, :], giF0[:])
            giF1_t = pool.tile([128, 16], I16, tag="giF1")
            A.dma_start(giF1_t[:, :], giF1[:])
            giM_t = pool.tile([128, 32], I16, tag="giM")
            A.dma_start(giM_t[:, :], giM[:])
            WYF_t = cin("WYF", WYF, (128, 256))
            WYM_t = cin("WYM", WYM, (64, 256))
            WXF_t = cin("WXF", WXF, (128, 128))
            C6_t = cin("C6", C6, (128, 48))
            SXYD_t = cin("SXYD", SXYD, (128, 128))
            G5_t = cin("G5", G5, (5, 128))
            GR5_t = cin("GR5", GR5, (5, 128))
            IDN_t = cin("IDN", IDN, (128, 128))
            EQ_t = cin("EQ", EQ, (6, 288))
            IEYE_t = pool.tile([8, 9], F32, tag="IEYE")
            V.memset(IEYE_t[:, :], 0.0)
            V.memset(IEYE_t[:, 0:9:4], 1.0)
            ONESC_t = pool.tile([128, 1], F32, tag="ONESC")
            V.memset(ONESC_t[:, :], 1.0 / NPTS)
            ONESR_t = pool.tile([1, 128], F32, tag="ONESR")
            V.memset(ONESR_t[:, :], 1.0)
            # prefetch the ACT function table (Sqrt/Abs) off the critical path
            ACTJ = pool.tile([8, 2], F32, tag="ACTJ")
            V.memset(ACTJ[:, :], 1.0)
            A.activation(ACTJ[:, 0:1], ACTJ[:, 1:2], ACTF.Sqrt)
            A.activation(ACTJ[:, 1:2], ACTJ[:, 0:1], ACTF.Abs)
            # ---------------- row gathers (flow halves first) ----------------
            tF = pool.tile([128, 4, 1152], F32)   # [p=(kd,bc')][s][pair row]
            tM = pool.tile([128, 4, 576], F32)    # [p=(a,kd,b)][s][row]
            for h, gi_t in ((0, giF0_t), (1, giF1_t)):
                G.dma_gather(
                    out_ap=tF[:, 2 * h : 2 * h + 2, :],
                    in_ap=_rows_view(flow.ap(), 2 * BPC * HF - 2, 1152),
                    idxs_ap=gi_t[:, :],
                    num_idxs=256,
                    num_idxs_reg=256,
                    elem_size=1152,
                    elem_step=WF,
                    queue_num=0,
                )
            G.dma_gather(
                out_ap=tM[:, :, :],
                in_ap=_rows_view(mask.ap(), BPC * HF, 576),
                idxs_ap=giM_t[:, :],
                num_idxs=512,
                num_idxs_reg=512,
                elem_size=576,
                queue_num=1,
            )

            # ------- column select + bilinear interp, per flow half -------
            GxF = pool.tile([128, 2, 4, 32, 2], F32)  # [a][s][i][c2]
            GxM = pool.tile([128, 4, 32, 2], F32)     # [s][i][c2]
            tFv = tF[:, :, :].rearrange("p s (a w) -> p s a w", a=2)
            dF = pool.tile([128, 256], F32)
            VFt = pool.tile([128, 256], F32)
            dcF = pool.tile([128, 128], F32)
            sampF = pool.tile([128, 128], F32)
            for h in range(2):
                sl = slice(2 * h, 2 * h + 2)
                for (i0, L, st) in cc.segs:
                    base = int(cc.x0[i0])
                    for c2 in (0, 1):
                        src_ = tFv[:, sl, :, base + c2 : base + c2 + (L - 1) * st + 1 : st]
                        dst = GxF[:, :, sl, i0 : i0 + L, c2].transpose([0, 2, 1, 3])
                        V.tensor_copy(dst, src_)
                # rows: V = G0 + (G1-G0)*wy   (per-half views, 128 free)
                g0 = GxF[:, 0, sl, :, :].rearrange("p s i c -> p (s i c)")
                g1 = GxF[:, 1, sl, :, :].rearrange("p s i c -> p (s i c)")
                dh = dF[:, 128 * h : 128 * h + 128]
                vh = VFt[:, 128 * h : 128 * h + 128]
                wyh = WYF_t[:, 128 * h : 128 * h + 128]
                V.tensor_tensor(out=dh, in0=g1, in1=g0, op=ALU.subtract)
                V.tensor_tensor(out=dh, in0=dh, in1=wyh, op=ALU.mult)
                V.tensor_tensor(out=vh, in0=dh, in1=g0, op=ALU.add)
                # cols: samp = V0 + (V1-V0)*wx
                vv4 = vh.rearrange("p (s i c) -> p s i c", s=2, i=32, c=2)
                d3 = dcF[:, 64 * h : 64 * h + 64].rearrange("p (s i) -> p s i", s=2)
                s3 = sampF[:, 64 * h : 64 * h + 64].rearrange("p (s i) -> p s i", s=2)
                wx3 = WXF_t[:, 64 * h : 64 * h + 64].rearrange("p (s i) -> p s i", s=2)
                V.tensor_tensor(out=d3, in0=vv4[:, :, :, 1], in1=vv4[:, :, :, 0], op=ALU.subtract)
                V.tensor_tensor(out=d3, in0=d3, in1=wx3, op=ALU.mult)
                V.tensor_tensor(out=s3, in0=d3, in1=vv4[:, :, :, 0], op=ALU.add)
            # mask (single gather): select, then lerp across partition halves
            for (i0, L, st) in cc.segs:
                base = int(cc.x0[i0])
                for c2 in (0, 1):
                    srcm = tM[:, :, base + c2 : base + c2 + (L - 1) * st + 1 : st]
                    V.tensor_copy(GxM[:, :, i0 : i0 + L, c2], srcm)
            GxM2 = GxM[:, :, :, :].rearrange("p s i c -> p (s i c)")
            dM = pool.tile([64, 256], F32)
            VMt = pool.tile([64, 256], F32)
            GxMhi = pool.tile([64, 256], F32)
            V.tensor_copy(GxMhi[:, :], GxM2[64:128, :])
            V.tensor_tensor(out=dM[:, :], in0=GxMhi[:, :], in1=GxM2[0:64, :], op=ALU.subtract)
            V.tensor_tensor(out=dM[:, :], in0=dM[:, :], in1=WYM_t[:, :], op=ALU.mult)
            V.tensor_tensor(out=VMt[:, :], in0=dM[:, :], in1=GxM2[0:64, :], op=ALU.add)
            VMv = VMt[:, :].rearrange("p (s i c) -> p s i c", s=4, i=32, c=2)
            dcM = pool.tile([64, 128], F32)
            sampM = pool.tile([64, 128], F32)
            dm3 = dcM[:, :].rearrange("p (s i) -> p s i", s=4)
            sm3 = sampM[:, :].rearrange("p (s i) -> p s i", s=4)
            wxm = WXF_t[0:64, :].rearrange("p (s i) -> p s i", s=4)
            V.tensor_tensor(out=dm3, in0=VMv[:, :, :, 1], in1=VMv[:, :, :, 0], op=ALU.subtract)
            V.tensor_tensor(out=dm3, in0=dm3, in1=wxm, op=ALU.mult)
            V.tensor_tensor(out=sm3, in0=dm3, in1=VMv[:, :, :, 0], op=ALU.add)

            # ---- transpose to points-on-partitions, fused with image-coord
            # ---- scaling and grid offsets: PQs = samp^T * diag(sxy) + grid
            psF = psp.tile([128, 128], F32)
            psM = psp.tile([128, 64], F32)
            T.matmul(psF[:, :], sampF[:, :], SXYD_t[:, :], start=True, stop=False)
            T.matmul(psF[:, :], G5_t[:, :], GR5_t[:, :], start=False, stop=True)
            T.transpose(psM[:, :], sampM[:, :], IDN_t[0:64, 0:64])
            PQs = pool.tile([128, 128], F32)  # dst img coords [pl][t 8][c 2][b 8]
            SM = pool.tile([128, 64], F32)    # mask sample    [pl][t 8][b 8]
            V.tensor_copy(PQs[:, :], psF[:, :])
            V.tensor_copy(SM[:, :], psM[:, :])

            # ---------------- Hartley stats ----------------
            psSum = psp.tile([1, 128], F32, tag="pss")
            T.matmul(psSum[:, :], ONESC_t[:, :], PQs[:, :], start=True, stop=True)
            SRow = pool.tile([1, 128], F32)
            V.tensor_copy(SRow[:, :], psSum[:, :])
            MRow = pool.tile([1, 16], F32)   # [c 2][b 8] means
            V.tensor_reduce(
                out=MRow[:, :].rearrange("o (g b) -> o g b", g=2, b=8),
                in_=SRow[:, :].rearrange("o (t g b) -> o g b t", g=2, t=8, b=8),
                axis=mybir.AxisListType.X,
                op=ALU.add,
            )
            psMB = psp.tile([128, 16], F32, tag="pss")
            T.matmul(psMB[:, :], ONESR_t[:, :], MRow[:, :], start=True, stop=True)
            MB = pool.tile([128, 16], F32)
            V.tensor_copy(MB[:, :], psMB[:, :])

            CXY = pool.tile([128, 128], F32)  # centered dst [t][c][b]
            SQ = pool.tile([128, 128], F32)
            R2 = pool.tile([128, 64], F32)
            SQR = pool.tile([128, 64], F32)
            mbv = MB[:, :].rearrange("p (c b) -> p c b", c=2, b=8).unsqueeze(1)
            V.tensor_tensor(out=CXY[:, :].rearrange("p (t c b) -> p t c b", t=8, c=2, b=8),
                            in0=PQs[:, :].rearrange("p (t c b) -> p t c b", t=8, c=2, b=8),
                            in1=mbv.broadcast_to([128, 8, 2, 8]), op=ALU.subtract)
            V.tensor_tensor(out=SQ[:, :], in0=CXY[:, :], in1=CXY[:, :], op=ALU.mult)
            sq3 = SQ[:, :].rearrange("p (t c b) -> p c t b", t=8, c=2, b=8)
            V.tensor_tensor(out=R2[:, :].rearrange("p (t b) -> p t b", t=8),
                            in0=sq3[:, 0, :, :], in1=sq3[:, 1, :, :], op=ALU.add)
            A.activation(SQR[:, :], R2[:, :], ACTF.Sqrt)
            psSq = psp.tile([1, 64], F32, tag="pss")
            T.matmul(psSq[:, :], ONESC_t[:, :], SQR[:, :], start=True, stop=True)
            SqRow = pool.tile([1, 64], F32)
            V.tensor_copy(SqRow[:, :], psSq[:, :])
            sRow = pool.tile([1, 8], F32)
            V.tensor_reduce(
                out=sRow[:, :].unsqueeze(1),
                in_=SqRow[:, :].rearrange("o (t b) -> o b t", t=8),
                axis=mybir.AxisListType.X,
                op=ALU.add,
            )
            V.tensor_scalar(out=sRow[:, :], in0=sRow[:, :],
                            scalar1=1.0 / math.sqrt(2.0), op0=ALU.mult,
                            scalar2=1e-8, op1=ALU.max)
            IR24 = pool.tile([1, 24], F32)   # [inv | inv | inv^2]
            V.reciprocal(IR24[:, 0:8], sRow[:, :])
            V.tensor_copy(IR24[:, 8:16], IR24[:, 0:8])
            V.tensor_tensor(out=IR24[:, 16:24], in0=IR24[:, 0:8], in1=IR24[:, 0:8],
                            op=ALU.mult)

            # ---------------- D = [w, w*cx, w*cy, w*r2] (unnormalized) -----
            D = pool.tile([128, 256], F32)    # [pl][t 8][q 4][b 8]
            Dv = D[:, :].rearrange("p (t q b) -> p q t b", q=4, b=8)
            V.tensor_scalar(out=Dv[:, 0, :, :],
                            in0=SM[:, :].rearrange("p (t b) -> p t b", t=8),
                            scalar1=0.0, op0=ALU.max, scalar2=None)  # w
            d12 = D[:, :].rearrange("p (t q b) -> p t q b", q=4, b=8)[:, :, 1:3, :]
            cxy12 = CXY[:, :].rearrange("p (t c b) -> p t c b", t=8, c=2, b=8)
            wb2 = Dv[:, 0, :, :].unsqueeze(2).broadcast_to([128, 8, 2, 8])
            V.tensor_tensor(out=d12, in0=cxy12, in1=wb2, op=ALU.mult)
            V.tensor_tensor(out=Dv[:, 3, :, :],
                            in0=R2[:, :].rearrange("p (t b) -> p t b", t=8),
                            in1=Dv[:, 0, :, :], op=ALU.mult)

            # ---------------- moments: M = C^T D ----------------
            psMom = psp.tile([6, 32], F32)
            for t in range(8):
                T.matmul(psMom[:, :], C6_t[:, 6 * t : 6 * t + 6],
                         D[:, 32 * t : 32 * t + 32], start=(t == 0), stop=(t == 7))
            Msb = pool.tile([6, 32], F32)
            V.tensor_copy(Msb[:, :], psMom[:, :])
            # normalize the moment columns: [wp, wq] *= 1/s ; [wr] *= 1/s^2
            psC6 = psp.tile([6, 24], F32, tag="pss")
            T.matmul(psC6[:, :], ONESR_t[0:1, 0:6], IR24[:, :], start=True, stop=True)
            SC6 = pool.tile([6, 24], F32)
            V.tensor_copy(SC6[:, :], psC6[:, :])
            V.tensor_tensor(out=Msb[:, 8:32], in0=Msb[:, 8:32], in1=SC6[:, :],
                            op=ALU.mult)

            # ---------------- assemble [A^T A | A^T b] via PE ----------------
            psA = psp.tile([72, 8], F32)
            for q in range(4):
                T.matmul(psA[:, :], EQ_t[:, 72 * q : 72 * q + 72],
                         Msb[0:6, 8 * q : 8 * q + 8], start=(q == 0), stop=(q == 3))
            AS = pool.tile([72, 8], F32)
            V.tensor_copy(AS[:, :], psA[:, :])
            psAT = psp.tile([8, 72], F32, tag="pss")
            T.transpose(psAT[:, :], AS[:, :], IDN_t[0:72, 0:72])
            AUG = pool.tile([8, 72], F32)
            V.tensor_copy(AUG[:, :], psAT[:, :])
            V.tensor_scalar(out=AUG[:, 0:71:10], in0=AUG[:, 0:71:10],
                            scalar1=EPS, op0=ALU.add, scalar2=None)

            # ---------------- per-batch scalars to partitions --------------
            PR = pool.tile([1, 128], F32)
            V.tensor_copy(PR[:, 0:8], MRow[:, 0:8])
            V.tensor_copy(PR[:, 32:40], MRow[:, 8:16])
            V.tensor_copy(PR[:, 64:72], sRow[:, :])
            psSC = psp.tile([128, 1], F32, tag="pss")
            T.transpose(psSC[:, :], PR[:, :], IDN_t[0:1, 0:1])
            SC = pool.tile([128, 1], F32)
            V.tensor_copy(SC[:, :], psSC[:, :])
            SCC = pool.tile([8, 4], F32)
            V.tensor_copy(SCC[:, 0:1], SC[0:8, :])      # mx (dst mean x)
            V.tensor_copy(SCC[:, 1:2], SC[32:40, :])    # my
            V.tensor_copy(SCC[:, 2:3], SC[64:72, :])    # s_dst
            # support gate: AUG[2,2] = S1 + eps
            V.tensor_scalar(out=SCC[:, 3:4], in0=AUG[:, 20:21],
                            scalar1=NPTS * 1e-4 + EPS, op0=ALU.is_gt, scalar2=None)

            # ---------------- Gauss-Jordan ----------------
            RK = pool.tile([8, 9], F32)
            PIV = pool.tile([8, 1], F32)
            U8 = pool.tile([8, 72], F32)
            for k in range(8):
                w_ = 9 - k  # active columns k..8
                V.reciprocal(PIV[:, :], AUG[:, 9 * k + k : 9 * k + k + 1])
                V.tensor_scalar(out=RK[:, 0:w_], in0=AUG[:, 9 * k + k : 9 * k + 9],
                                scalar1=PIV[:, :], op0=ALU.mult, scalar2=None)
                fcol = AUG[:, k : 72 : 9].unsqueeze(2).broadcast_to([8, 8, w_])
                rkb = RK[:, 0:w_].unsqueeze(1).broadcast_to([8, 8, w_])
                ucols = U8[:, :].rearrange("p (r c) -> p r c", r=8)[:, :, 0:w_]
                acols = AUG[:, :].rearrange("p (r c) -> p r c", r=8)[:, :, k:9]
                V.tensor_tensor(out=ucols, in0=fcol, in1=rkb, op=ALU.mult)
                V.tensor_tensor(out=acols, in0=acols, in1=ucols, op=ALU.subtract)
                V.tensor_copy(AUG[:, 9 * k + k : 9 * k + 9], RK[:, 0:w_])

            # ---------------- denormalize + gate ----------------
            c_ = V.tensor_copy
            HN = pool.tile([8, 9], F32)
            c_(HN[:, 0:8], AUG[:, 8:72:9])
            V.memset(HN[:, 8:9], 1.0)
            mx_sc, my_sc = SCC[:, 0:1], SCC[:, 1:2]
            s_sc, g_sc = SCC[:, 2:3], SCC[:, 3:4]
            T1 = pool.tile([8, 9], F32)
            H1 = pool.tile([8, 9], F32)
            V.tensor_scalar(out=T1[:, 0:3], in0=HN[:, 0:3], scalar1=s_sc, op0=ALU.mult, scalar2=None)
            V.scalar_tensor_tensor(out=H1[:, 0:3], in0=HN[:, 6:9], scalar=mx_sc,
                                   in1=T1[:, 0:3], op0=ALU.mult, op1=ALU.add)
            V.tensor_scalar(out=T1[:, 3:6], in0=HN[:, 3:6], scalar1=s_sc, op0=ALU.mult, scalar2=None)
            V.scalar_tensor_tensor(out=H1[:, 3:6], in0=HN[:, 6:9], scalar=my_sc,
                                   in1=T1[:, 3:6], op0=ALU.mult, op1=ALU.add)
            c_(H1[:, 6:9], HN[:, 6:9])
            H2 = pool.tile([8, 9], F32)
            H1v = H1[:, :].rearrange("p (r c) -> p r c", r=3)
            H2v = H2[:, :].rearrange("p (r c) -> p r c", r=3)
            V.tensor_scalar(out=H2v[:, :, 0:2], in0=H1v[:, :, 0:2], scalar1=cc.a_ts, op0=ALU.mult, scalar2=None)
            T2 = pool.tile([8, 3], F32)
            T3 = pool.tile([8, 3], F32)
            V.tensor_scalar(out=T2[:, :], in0=H1[:, 0:9:3], scalar1=cc.c_ts, op0=ALU.mult, scalar2=None)
            V.scalar_tensor_tensor(out=T3[:, :], in0=H1[:, 1:9:3], scalar=cc.d_ts,
                                   in1=T2[:, :], op0=ALU.mult, op1=ALU.add)
            V.tensor_tensor(out=H2[:, 2:9:3], in0=T3[:, :], in1=H1[:, 2:9:3], op=ALU.add)
            ABSD = pool.tile([8, 1], F32)
            SGN = pool.tile([8, 1], F32)
            DEN = pool.tile([8, 1], F32)
            RECD = pool.tile([8, 1], F32)
            A.activation(ABSD[:, :], H2[:, 8:9], ACTF.Abs)
            V.tensor_scalar(out=ABSD[:, :], in0=ABSD[:, :], scalar1=1e-8, op0=ALU.max, scalar2=None)
            V.tensor_scalar(out=SGN[:, :], in0=H2[:, 8:9], scalar1=0.0, op0=ALU.is_lt,
                            scalar2=-2.0, op1=ALU.mult)
            V.tensor_scalar(out=SGN[:, :], in0=SGN[:, :], scalar1=1.0, op0=ALU.add, scalar2=None)
            V.tensor_tensor(out=DEN[:, :], in0=ABSD[:, :], in1=SGN[:, :], op=ALU.mult)
            V.reciprocal(RECD[:, :], DEN[:, :])
            V.tensor_scalar(out=H2[:, :], in0=H2[:, :], scalar1=RECD[:, :], op0=ALU.mult, scalar2=None)
            IG = pool.tile([8, 1], F32)
            TI = pool.tile([8, 9], F32)
            OUTt = pool.tile([8, 9], F32)
            V.tensor_scalar(out=IG[:, :], in0=g_sc, scalar1=-1.0, op0=ALU.mult,
                            scalar2=1.0, op1=ALU.add)
            V.tensor_scalar(out=TI[:, :], in0=IEYE_t[:, :], scalar1=IG[:, :], op0=ALU.mult, scalar2=None)
            V.scalar_tensor_tensor(out=OUTt[:, :], in0=H2[:, :], scalar=g_sc,
                                   in1=TI[:, :], op0=ALU.mult, op1=ALU.add)
            S.dma_start(Hout.ap().rearrange("b r c -> b (r c)"), OUTt[:, :])

    nc.compile()
    return nc


# ---------------------------------------------------------------------------
# host wrapper
# ---------------------------------------------------------------------------

_CACHE = {}


def _get(img_h, img_w):
    key = (int(img_h), int(img_w))
    if key not in _CACHE:
        cc = _Consts(*key)
        _CACHE[key] = (cc, _build_program(cc))
    return _CACHE[key]


def _in_maps(cc, flow, mask):
    flow = np.ascontiguousarray(flow, np.float32)
    mask = np.ascontiguousarray(mask, np.float32)
    maps = []
    for c in range(NCORES):
        maps.append({
            "flow": flow[c * BPC : (c + 1) * BPC],
            "mask": mask[c * BPC : (c + 1) * BPC],
            "giF0": cc.giF[0], "giF1": cc.giF[1], "giM": cc.giM,
            "WYF": cc.WYF, "WYM": cc.WYM, "WXF": cc.WXF,
            "C6": cc.C6, "SXYD": cc.SXYD, "G5": cc.G5, "GR5": cc.GR5,
            "IDN": cc.IDN, "EQ": cc.EQ,
        })
    return maps


def run(flow, mask, img_h, img_w, trace=False, **spmd_kwargs):
    cc, nc = _get(img_h, img_w)
    res = bass_utils.run_bass_kernel_spmd(
        nc, _in_maps(cc, flow, mask), list(range(NCORES)), trace=trace, **spmd_kwargs
    )
    out = np.concatenate([res.results[c]["H"] for c in range(NCORES)], axis=0)
    return out.astype(np.float32), res


def kernel(flow, mask, img_h, img_w):
    out, _ = run(flow, mask, img_h, img_w)
    return out

